# revision 24
# baseline (speedup 1.0000x reference)
"""Trainium2 Bass kernel for nn_CustomEmbed (ConvNeXt-style embed stack).

Data-parallel over batch: 8 images per NeuronCore x 8 cores.
Patchify convs (conv1/2/3) are bf16 PE matmuls with a uniform 96+96
channel split so K,M always round to the (128,128) PE tiling mode (no
mode-switch drains). Region convs (3x3 SAME on 7x7 tiles, per-group
weights) run as fp8e4m3 DoubleRow matmuls: the 192-channel contraction
is fused into one matmul per tap via the row-pair mode (weights host-
scaled x64 into fp8 normal range, compensated by activation scale=1/64).
Each group's input lives in a zero-padded 9x8 fp8 tile so all 9 taps are
full N=448 matmuls; per-tap results land in a flat 452-slot PSUM bank at
a constant offset (1-dx), edge garbage falling into never-read slots.
BN folded into weights/bias on host; GELU via ScalarE LUT (erf-exact);
residual adds on VectorE. All intermediates live in SBUF.
"""
import numpy as np
import ml_dtypes

import concourse.bass as bass
import concourse.tile as tile
from concourse import bacc, mybir
from concourse.ap import AP
from concourse.bass_utils import run_bass_kernel_spmd

AF = mybir.ActivationFunctionType
PM = mybir.MatmulPerfMode
dt = mybir.dt
BF16 = ml_dtypes.bfloat16

EPS = 1e-5
B = 64
NCORE = 8
IMG = B // NCORE          # 8 images per core
C4 = 192
ED = 768
H = 96                    # channel half
WSCALE = 64.0             # fp8 weight pre-scale (into e4m3 normal range)

TAPS = [(0, 0)] + [(dy, dx) for dy in (-1, 0, 1) for dx in (-1, 0, 1)
                   if (dy, dx) != (0, 0)]          # center first (PSUM start)
PHASES = [(0, 0), (0, 1), (1, 0), (1, 1)]          # p = 2*sy + sx


# ---------------------------------------------------------------- host prep

def _fold(w, g, b, m, v, co_axis):
    """Fold inference BN into conv weight + bias. w scaled along co_axis."""
    inv = (g / np.sqrt(v + EPS)).astype(np.float32)
    bias = (b - m * inv).astype(np.float32)
    shape = [1] * w.ndim
    shape[co_axis] = -1
    return (w.astype(np.float32) * inv.reshape(shape)).astype(np.float32), bias


def prep_weights(inp):
    """Returns dict of device-ready weight arrays (shared across cores)."""
    E4 = dt.np(dt.float8e4)
    out = {}
    w1, b1 = _fold(inp["conv1_w"], inp["bn1_g"], inp["bn1_b"], inp["bn1_m"],
                   inp["bn1_v"], 0)                      # (192, 3, 4, 4)
    w1t = w1.transpose(1, 2, 3, 0).reshape(48, C4)
    w1p = np.zeros((H, C4), np.float32)
    w1p[:48] = w1t
    out["w1"] = w1p.astype(BF16)                         # (96, 192)
    out["b1"] = np.ascontiguousarray(b1.reshape(2, H).T)  # (96, 2)

    # region weights: (G, co, ci, ky, kx) -> (G, Ki=96, Ko=2, tap, co) fp8
    for nm, G in (("r1", 64), ("r2", 16)):
        w = inp[f"{nm}_w"].astype(np.float32)
        inv = (inp[f"{nm}_g"] / np.sqrt(inp[f"{nm}_v"] + EPS)).astype(np.float32)
        bias = (inp[f"{nm}_b"] - inp[f"{nm}_m"] * inv).astype(np.float32)
        w = w * inv[:, :, None, None, None]
        wt = w.transpose(0, 2, 3, 4, 1).reshape(G, C4, 9, C4) * WSCALE
        out[f"{nm}w"] = np.ascontiguousarray(
            np.stack([wt[:, :H], wt[:, H:]], axis=2)).astype(E4)
        out[f"{nm}b"] = np.ascontiguousarray(
            bias.T.reshape(2, H, G).transpose(1, 0, 2))   # (96, 2, G)

    w2, b2 = _fold(inp["conv2_w"], inp["bn2_g"], inp["bn2_b"], inp["bn2_m"],
                   inp["bn2_v"], 0)                      # (192, 192, 2, 2)
    w2t = w2.transpose(1, 2, 3, 0).reshape(C4, 4, C4) * WSCALE
    out["w2"] = np.ascontiguousarray(
        np.stack([w2t[:H], w2t[H:]], axis=1)).astype(E4)  # (96, 2, 4, 192)
    out["b2"] = np.ascontiguousarray(b2.reshape(2, H).T)

    w3, b3 = _fold(inp["conv3_w"], inp["bn3_g"], inp["bn3_b"], inp["bn3_m"],
                   inp["bn3_v"], 0)                      # (768, 192, 2, 2)
    w3t = w3.transpose(1, 2, 3, 0).reshape(C4, 4, ED)
    out["w3"] = np.ascontiguousarray(
        np.stack([w3t[:H], w3t[H:]], axis=0)).astype(BF16)  # (2, 96, 4, 768)
    out["b3"] = np.ascontiguousarray(b3.reshape(6, 128).T)  # (128, 6)
    return out


def prep_a1(x_core):
    """x (IMG,3,224,224) fp32 -> a1 [96, 64 groups, IMG*49] bf16 im2col
    (rows 48..95 zero so conv1 K rounds to the 128-row PE mode)."""
    i = x_core.shape[0]
    t = x_core.reshape(i, 3, 8, 7, 4, 8, 7, 4)     # (i, c, gy, y, dy, gx, x, dx)
    t = t.transpose(1, 4, 7, 2, 5, 0, 3, 6)        # (c, dy, dx, gy, gx, i, y, x)
    a = np.zeros((H, 64, i * 49), np.float32)
    a[:48] = t.reshape(48, 64, i * 49)
    return np.ascontiguousarray(a).astype(BF16)


# ------------------------------------------------------------- device build

def build_program():
    nc = bacc.Bacc("TRN2", target_bir_lowering=False)

    a1_d = nc.declare_dram_parameter("a1", [H, 64, IMG * 49], dt.bfloat16, isOutput=False)
    w1_d = nc.declare_dram_parameter("w1", [H, C4], dt.bfloat16, isOutput=False)
    b1_d = nc.declare_dram_parameter("b1", [H, 2], dt.float32, isOutput=False)
    r1w_d = nc.declare_dram_parameter("r1w", [64, H, 2, 9, C4], dt.float8e4, isOutput=False)
    r1b_d = nc.declare_dram_parameter("r1b", [H, 2, 64], dt.float32, isOutput=False)
    w2_d = nc.declare_dram_parameter("w2", [H, 2, 4, C4], dt.float8e4, isOutput=False)
    b2_d = nc.declare_dram_parameter("b2", [H, 2], dt.float32, isOutput=False)
    r2w_d = nc.declare_dram_parameter("r2w", [16, H, 2, 9, C4], dt.float8e4, isOutput=False)
    r2b_d = nc.declare_dram_parameter("r2b", [H, 2, 16], dt.float32, isOutput=False)
    w3_d = nc.declare_dram_parameter("w3", [2, H, 4, ED], dt.bfloat16, isOutput=False)
    b3_d = nc.declare_dram_parameter("b3", [128, 6], dt.float32, isOutput=False)
    out_d = nc.declare_dram_parameter("out3", [ED, IMG * 196], dt.float32,
                                      isOutput=True)

    with tile.TileContext(nc) as tc:
        with (
            tc.tile_pool(name="pers", bufs=1) as pp,
            tc.tile_pool(name="const", bufs=1) as cp,
            tc.tile_pool(name="wpool", bufs=3) as wp,
            tc.tile_pool(name="io", bufs=4) as io,
            tc.tile_pool(name="ps", bufs=2, space="PSUM") as ps,
        ):
            # persistent SBUF intermediates (channel halves lo=0..95, hi=96..191)
            h1b8 = pp.tile([H, 2, IMG, 56, 56], dt.float8e4, name="h1b8")
            h2L = pp.tile([H, IMG, 28, 28], dt.bfloat16, name="h2L")
            h2H = pp.tile([H, IMG, 28, 28], dt.bfloat16, name="h2H")
            h2bL = pp.tile([H, IMG, 28, 28], dt.bfloat16, name="h2bL")
            h2bH = pp.tile([H, IMG, 28, 28], dt.bfloat16, name="h2bH")
            # 3-slot ring of zero-padded 9x9 fp8 region input tiles
            # warm the Gelu LUT during the initial DMAs so the first real
            # activation doesn't pay the table load
            scr = pp.tile([H, 2], dt.float32, name="scr")
            nc.vector.memset(scr[:], 0)
            nc.scalar.activation(scr[:, 1:2], scr[:, 0:1], AF.Gelu)
            t8 = pp.tile([H, 4, 2, IMG, 9, 8], dt.float8e4, name="t8")
            nc.vector.memset(t8[:], 0)

            # ---- resident constants
            w1t = cp.tile([H, C4], dt.bfloat16, name="w1t")
            nc.sync.dma_start(w1t[:], w1_d[:])
            b1t = cp.tile([H, 2], dt.float32, name="b1t")
            nc.sync.dma_start(b1t[:], b1_d[:])
            r1bt = cp.tile([H, 2, 64], dt.float32, name="r1bt")
            nc.sync.dma_start(r1bt[:], r1b_d[:])
            w28 = cp.tile([H, 2, 4, C4], dt.float8e4, name="w28")
            nc.scalar.dma_start(w28[:], w2_d[:])
            b2t = cp.tile([H, 2], dt.float32, name="b2t")
            nc.sync.dma_start(b2t[:], b2_d[:])
            r2bt = cp.tile([H, 2, 16], dt.float32, name="r2bt")
            nc.scalar.dma_start(r2bt[:], r2b_d[:])
            w3L = cp.tile([H, 4, ED], dt.bfloat16, name="w3L")
            nc.scalar.dma_start(w3L[:], w3_d[0])
            w3H = cp.tile([H, 4, ED], dt.bfloat16, name="w3H")
            nc.scalar.dma_start(w3H[:], w3_d[1])
            b3t = cp.tile([128, 6], dt.float32, name="b3t")
            nc.scalar.dma_start(b3t[:], b3_d[:])

            def region_dr(slot, w8, bias_col, g, dstL, dstH, resL, resH,
                          y0, x0):
                """9-tap DoubleRow region conv on t8 slot -> dst windows."""
                for mi in range(2):
                    pt = ps.tile([H, 452], dt.float32, name="pr%d" % mi,
                                 tag="r%d" % mi)
                    base = pt[:, :]
                    for imm, (dy, dx) in enumerate(TAPS):
                        tap = 3 * (dy + 1) + (dx + 1)
                        ya, yb = max(1, 1 + dy), min(8, 8 + dy)
                        j0 = (1 - dx) + (8 if dy < 0 else 0)
                        out_ap = AP(base.tensor, base.offset + j0,
                                    [list(base.ap[0]), [56, IMG],
                                     [1, 8 * (yb - ya)]])
                        nc.tensor.matmul(
                            out_ap,
                            w8[:, :, tap, H * mi:H * mi + H],
                            t8[:, slot, :, :, ya:yb, :],
                            start=(imm == 0), stop=(imm == 8),
                            perf_mode=PM.DoubleRow)
                    rd = AP(base.tensor, base.offset + 1,
                            [list(base.ap[0]), [56, IMG], [8, 7], [1, 7]])
                    gt = io.tile([H, IMG, 7, 7], dt.bfloat16, name="gt%d" % mi)
                    nc.scalar.activation(gt[:], rd, AF.Gelu,
                                         bias=bias_col(mi, g),
                                         scale=1.0 / WSCALE)
                    dst = (dstL, dstH)[mi]
                    res = (resL, resH)[mi]
                    nc.vector.tensor_add(
                        dst[:, :, y0:y0 + 7, x0:x0 + 7], gt[:], res)

            def conv2_unit(i, hh):
                """One conv2 output block: (image i, row half hh), both
                m-halves. 8 fp8 DoubleRow matmuls + 2 acts."""
                r0, rend = 28 * hh, 28 * hh + 28
                for mi, tg in ((0, "c0"), (1, "c1")):
                    pt = ps.tile([H, 14, 28], dt.float32,
                                 name="psC" + tg, tag=tg)
                    for p, (sy, sx) in enumerate(PHASES):
                        nc.tensor.matmul(
                            pt[:],
                            w28[:, :, p, H * mi:H * mi + H],
                            h1b8[:, :, i, r0 + sy:rend:2, sx::2],
                            start=(p == 0), stop=(p == 3),
                            perf_mode=PM.DoubleRow)
                    dst = (h2L, h2H)[mi]
                    nc.scalar.activation(dst[:, i, 14 * hh:14 * hh + 14, :],
                                         pt[:], AF.Gelu,
                                         bias=b2t[:, mi:mi + 1],
                                         scale=1.0 / WSCALE)

            # ---- stage AB: fused conv1 + region1 per group -> h1bL/h1bH
            # conv1 runs 2 groups ahead of the region stage (software
            # pipeline) so the PE always has conv matmuls to fill region
            # dependency latency. conv2 (i, hh=0) units are woven into the
            # region1 tail (their h1b rows are ready after g=31): the bf16
            # conv matmuls hide the region DoubleRow LDWEIGHTS time.
            tLHs = {}

            def conv1_part(g):
                slot = g % 4
                a1t = io.tile([H, IMG * 49], dt.bfloat16, name="a1t")
                nc.sync.dma_start(a1t[:], a1_d[:, g, :])
                tLH = io.tile([H, 2, IMG, 7, 7], dt.bfloat16, name="tLH")
                ps0 = ps.tile([H, IMG * 49], dt.float32, name="ps0", tag="c0")
                nc.tensor.matmul(ps0[:], w1t[:, 0:H], a1t[:],
                                 start=True, stop=True)
                nc.scalar.activation(tLH[:, 0], ps0[:], AF.Gelu,
                                     bias=b1t[:, 0:1])
                ps1 = ps.tile([H, IMG * 49], dt.float32, name="ps1", tag="c1")
                nc.tensor.matmul(ps1[:], w1t[:, H:C4], a1t[:],
                                 start=True, stop=True)
                nc.scalar.activation(tLH[:, 1], ps1[:], AF.Gelu,
                                     bias=b1t[:, 1:2])
                nc.vector.tensor_copy(t8[:, slot, :, :, 1:8, 0:7], tLH[:])
                tLHs[g] = tLH

            conv1_part(0)
            conv1_part(1)
            for g in range(64):
                gy, gx = divmod(g, 8)
                if g + 2 < 64:
                    conv1_part(g + 2)
                w8 = wp.tile([H, 2, 9, C4], dt.float8e4, name="w8")
                nc.gpsimd.dma_start(w8[:], r1w_d[g])
                tLH = tLHs.pop(g)
                region_dr(g % 4, w8, lambda mi, gg: r1bt[:, mi, gg:gg + 1], g,
                          h1b8[:, 0], h1b8[:, 1], tLH[:, 0], tLH[:, 1],
                          7 * gy, 7 * gx)
                if 33 <= g <= 47 and g % 2 == 1:
                    conv2_unit((g - 33) // 2, 0)

            # ---- stage C remainder + stage D: region2 -> h2bL/h2bH (SBUF)
            # conv2 (i, hh=1) units weave into the first region2 groups.
            for g in range(16):
                gy, gx = divmod(g, 4)
                slot = (64 + g) % 4
                y0, x0 = 7 * gy, 7 * gx
                w8 = wp.tile([H, 2, 9, C4], dt.float8e4, name="w8")
                nc.gpsimd.dma_start(w8[:], r2w_d[g])
                nc.vector.tensor_copy(t8[:, slot, 0, :, 1:8, 0:7],
                                      h2L[:, :, y0:y0 + 7, x0:x0 + 7])
                nc.vector.tensor_copy(t8[:, slot, 1, :, 1:8, 0:7],
                                      h2H[:, :, y0:y0 + 7, x0:x0 + 7])
                region_dr(slot, w8, lambda mi, gg: r2bt[:, mi, gg:gg + 1], g,
                          h2bL, h2bH,
                          h2L[:, :, y0:y0 + 7, x0:x0 + 7],
                          h2H[:, :, y0:y0 + 7, x0:x0 + 7], y0, x0)
                if g < 8:
                    conv2_unit(g, 1)

            # ---- stage E: conv3 + BN -> out3 (DRAM)
            for ip in range(4):
                for j in range(6):
                    tg = ("c0", "c1")[j % 2]
                    psE = ps.tile([128, 2, 14, 14], dt.float32,
                                  name="psE" + tg, tag=tg)
                    imm = 0
                    for p, (sy, sx) in enumerate(PHASES):
                        for kh, src in ((0, h2bL), (1, h2bH)):
                            wt = (w3L, w3H)[kh]
                            nc.tensor.matmul(
                                psE[:],
                                wt[:, p, 128 * j:128 * j + 128],
                                src[:, 2 * ip:2 * ip + 2, sy::2, sx::2],
                                start=(imm == 0), stop=(imm == 7))
                            imm += 1
                    oE = io.tile([128, 2, 14, 14], dt.float32, name="oE")
                    nc.vector.tensor_scalar_add(oE[:], psE[:], b3t[:, j:j + 1])
                    nc.sync.dma_start(
                        out_d[128 * j:128 * j + 128,
                              392 * ip:392 * (ip + 1)],
                        oE[:])

    nc.compile()
    return nc


_NC_CACHE = {}


def _get_program():
    if "nc" not in _NC_CACHE:
        _NC_CACHE["nc"] = build_program()
    return _NC_CACHE["nc"]


def run(inputs, trace=False):
    """Returns (output, BassKernelResults)."""
    nc = _get_program()
    wts = prep_weights(inputs)
    x = np.asarray(inputs["x"], np.float32)
    in_maps = []
    for c in range(NCORE):
        m = dict(wts)
        m["a1"] = prep_a1(x[c * IMG:(c + 1) * IMG])
        in_maps.append(m)
    res = run_bass_kernel_spmd(nc, in_maps, list(range(NCORE)), trace=trace)
    # gather: per-core (768, IMG*196) -> (B, 196, 768)
    outs = [np.asarray(r["out3"]).reshape(ED, IMG, 196).transpose(1, 2, 0)
            for r in res.results]
    full = np.ascontiguousarray(np.concatenate(outs, axis=0), dtype=np.float32)
    return full, res


def kernel(**inputs):
    return run(inputs)[0]
